# revision 1
# baseline (speedup 1.0000x reference)
"""MQA attention kernel for Trainium2 (8 NeuronCores, Bass/Tile).

Problem: Q [2,16,2048,64], K/V [2,1,2048,64] fp32, out = softmax(QK^T/8) V.

Sharding: 32 (batch, head) pairs over 8 cores -> 4 heads per core; each core
gets one batch's K/V (replicated across the 4 cores of that batch).

Per-core algorithm (S^T orientation so softmax reduction lands on the free dim
and PV needs no transposition of P):
  - K^T, Q^T built on-chip via PE transposes (d=64 on partitions, zero-padded
    to 128 so all matmuls contract over the full partition dim).
  - S^T[j, q] = (K Q^T) computed in fp32r matmuls (full-rate 4-byte dtype),
    PSUM bank per 128-row j-chunk.
  - exp(s/8) fused with PSUM->SBUF evacuation on the scalar engine (no max
    subtraction: scores/8 ~ N(0,1), exp never overflows fp32).
  - PV uses V augmented with a ones column: one matmul chain yields both
    O^T = V'^T P^T and the softmax denominators (row 64).
  - O'^T transposed back with PE, normalized with DVE reciprocal+mul, DMA out.

The q-rows are processed in an interleaved order (partition p holds rows
16p+c) so every DMA moves contiguous 4KB runs; the same rearrange on the
output store undoes the permutation.

Scheduling: K^T and head-0 Q^T transposes are interleaved right after the
input DMAs (first exp starts ~8us in); head h+1's Q^T transposes are hoisted
into head h's first q-block so the scalar engine never stalls at head
boundaries. PSUM budget (8 banks): 2x2 rotating score banks + 1 PV
accumulator + 2 transpose staging.

Measured on trn2 (NTFF profile, core 0): 180.4 us end-to-end, scalar engine
(exp, the only transcendental unit) ~80% busy = the roofline driver;
rel err vs fp64-ish jax reference: 1.9e-4 (fp32r matmuls).
"""

import numpy as np

import concourse.bass as bass
import concourse.mybir as mybir
import concourse.tile as tile
from concourse import bacc
from concourse.bass_utils import run_bass_kernel_spmd
from concourse.masks import make_identity

B, H, S, D = 2, 16, 2048, 64
N_CORES = 8
HPC = (B * H) // N_CORES  # heads per core = 4
P = 128
NJ = S // P               # 16 key chunks of 128
QB = 512                  # queries per block (= max fp32 matmul free dim)
NQB = S // QB             # 4 q-blocks per head
SCALE = 1.0 / float(D) ** 0.5
F32 = mybir.dt.float32
F32R = mybir.dt.float32r  # 4-byte matmul dtype, 2 cyc/row streaming
BF16 = mybir.dt.bfloat16
EXP_GRP = 2               # j-chunks (PSUM banks) per exp ACTIVATE group
ROW_TILE = False          # run QK^T matmul pairs in PE row-groups 0-63 / 64-127

_CACHED = {}
# Best measured config (profiled on HW); kernel() uses this.
DEFAULT_CFG = {}


def _build_module(reps=1, **cfg):
    nc = bacc.Bacc(None)
    q = nc.dram_tensor("q", [HPC, S, D], F32, kind="ExternalInput")
    k = nc.dram_tensor("k", [S, D], F32, kind="ExternalInput")
    v = nc.dram_tensor("v", [S, D], F32, kind="ExternalInput")
    o = nc.dram_tensor("o", [HPC, S, D], F32, kind="ExternalOutput")

    with tile.TileContext(nc) as tc:
        with tc.tile_pool(name="const", bufs=1) as cpool:
            identity = cpool.tile([P, P], F32)
            make_identity(nc, identity)

            kT = cpool.tile([P, S], F32R)
            nc.gpsimd.memset(kT[64:P, :].bitcast(mybir.dt.uint32), 0)
            vp = cpool.tile([P, NJ, D + 1], F32R)
            nc.gpsimd.memset(vp[:, :, D].bitcast(mybir.dt.uint32), 0x3F800000)
            qT_tiles = []
            for i in range(2):
                qTt = cpool.tile([P, S], F32R, name=f"qT{i}")
                nc.gpsimd.memset(qTt[64:P, :].bitcast(mybir.dt.uint32), 0)
                qT_tiles.append(qTt)

            # `reps` re-traces the whole per-core program body (for
            # differential wall-clock timing of the device work without RPC
            # dispatch overhead); the functional kernel uses reps=1.
            for rep in range(reps):
                _trace_body(nc, tc, q, k, v, o, identity, kT, vp, qT_tiles, **cfg)
    nc.compile()
    return nc


def _trace_body(
    nc, tc, q, k, v, o, identity, kT, vp, qT_tiles,
    exp_grp=None, row_tile=None, pt_bufs=2, sg_bufs=2, tr_bufs=2, pv_bf16=False,
):
    EXP_GRP = exp_grp if exp_grp is not None else globals()["EXP_GRP"]
    ROW_TILE = row_tile if row_tile is not None else globals()["ROW_TILE"]
    with (
        tc.tile_pool(name="natb", bufs=2) as npool,
        tc.tile_pool(name="workb", bufs=pt_bufs) as wpool,
        tc.tile_pool(name="psb", bufs=sg_bufs, space="PSUM") as pspool,
        tc.tile_pool(name="ps1b", bufs=1, space="PSUM") as ps1pool,
    ):
            def transpose_64(dst, src_nat, who):
                # PE-transpose 4 [128,64] chunks into one PSUM tile, then one
                # DVE cast into [64, 512] of the fp32r destination.
                for g in range(NJ // 4):
                    pst = ps1pool.tile(
                        [64, 4, P], F32, tag="tr", bufs=tr_bufs, name=f"pst_{who}{g}"
                    )
                    for t in range(4):
                        nc.tensor.transpose(
                            pst[:, t, :], src_nat[:, 4 * g + t, :], identity
                        )
                    nc.vector.tensor_copy(dst[0:64, 512 * g : 512 * (g + 1)], pst[:])

            def load_q(h):
                q_nat = npool.tile([P, NJ, D], F32, tag="nat", name=f"q_nat{h}")
                nc.sync.dma_start(q_nat[:], q[h].rearrange("(p c) d -> p c d", p=P))
                return q_nat

            def prep_qT(h, q_nat):
                qT = qT_tiles[h % 2]
                transpose_64(qT, q_nat, f"q{h}_")
                if ROW_TILE:
                    nc.sync.dma_start(qT[64:P, :], qT[0:64, :])
                return qT

            # ---- startup: K^T and head-0 Q^T, transposed interleaved ----
            k_nat = npool.tile([P, NJ, D], F32, tag="nat")
            nc.sync.dma_start(k_nat[:], k.rearrange("(p c) d -> p c d", p=P))
            q_nat_next = load_q(0)
            for g in range(NJ // 4):
                pstk = ps1pool.tile([64, 4, P], F32, tag="tr", bufs=tr_bufs, name=f"pst_k{g}")
                for t in range(4):
                    nc.tensor.transpose(pstk[:, t, :], k_nat[:, 4 * g + t, :], identity)
                nc.vector.tensor_copy(kT[0:64, 512 * g : 512 * (g + 1)], pstk[:])
                pstq = ps1pool.tile([64, 4, P], F32, tag="tr", bufs=tr_bufs, name=f"pst_q0{g}")
                for t in range(4):
                    nc.tensor.transpose(
                        pstq[:, t, :], q_nat_next[:, 4 * g + t, :], identity
                    )
                nc.vector.tensor_copy(
                    qT_tiles[0][0:64, 512 * g : 512 * (g + 1)], pstq[:]
                )
            if ROW_TILE:
                nc.sync.dma_start(kT[64:P, :], kT[0:64, :])
                nc.sync.dma_start(qT_tiles[0][64:P, :], qT_tiles[0][0:64, :])

            # ---- V' [128, 16, 65]: V plus a ones column (softmax denom) ----
            v_nat = npool.tile([P, NJ, D], F32, tag="nat", name="v_nat")
            nc.sync.dma_start(v_nat[:], v.rearrange("(p c) d -> p c d", p=P))
            if pv_bf16:
                vpb = wpool.tile([P, NJ, D + 1], BF16, tag="vpb", bufs=1, name="vpb")
                nc.gpsimd.memset(vpb[:, :, D], 1.0)
                nc.vector.tensor_copy(vpb[:, :, 0:D], v_nat[:])
                vp = vpb
            else:
                nc.vector.tensor_copy(vp[:, :, 0:D], v_nat[:])

            for h in range(HPC):
                qT = qT_tiles[h % 2]

                for qb in range(NQB):
                    qs = qT[:, QB * qb : QB * (qb + 1)]
                    # exp(S^T/8): j-chunk scores into PSUM, scalar engine
                    # evacuates each EXP_GRP-bank group with a fused exp.
                    pT = wpool.tile([P, NJ * QB], BF16 if pv_bf16 else F32R, tag="pT", name=f"pT{h}_{qb}")
                    if EXP_GRP == 3:
                        group_sizes = [3, 3, 3, 3, 2, 2]
                    else:
                        group_sizes = [EXP_GRP] * (NJ // EXP_GRP)
                    g_start = [sum(group_sizes[:i]) for i in range(len(group_sizes))]
                    for g, gsz in enumerate(group_sizes):
                        sg = pspool.tile(
                            [P, gsz, QB],
                            F32,
                            tag="sg",
                            name=f"sg{h}_{qb}_{g}",
                            padded_shape=[P, max(group_sizes), QB],
                        )
                        for i in range(gsz):
                            j = g_start[g] + i
                            if ROW_TILE:
                                half = slice(0, 64) if i % 2 == 0 else slice(64, P)
                                nc.tensor.matmul(
                                    sg[:, i, :],
                                    lhsT=kT[half, P * j : P * (j + 1)],
                                    rhs=qs[half, :],
                                    start=True,
                                    stop=True,
                                )
                            else:
                                nc.tensor.matmul(
                                    sg[:, i, :],
                                    lhsT=kT[:, P * j : P * (j + 1)],
                                    rhs=qs,
                                    start=True,
                                    stop=True,
                                )
                        nc.scalar.activation(
                            pT[:, QB * g_start[g] : QB * (g_start[g] + gsz)],
                            sg[:],
                            mybir.ActivationFunctionType.Exp,
                            scale=SCALE,
                        )
                    # O'^T [65, 512] = V'^T P^T accumulated over j-chunks
                    pv = ps1pool.tile([D + 1, QB], F32, tag="pv", name=f"pv{h}_{qb}")
                    for c in range(NJ):
                        nc.tensor.matmul(
                            pv[:],
                            lhsT=vp[:, c, :],
                            rhs=pT[:, QB * c : QB * (c + 1)],
                            start=(c == 0),
                            stop=(c == NJ - 1),
                        )
                    oev = wpool.tile([D + 1, QB], F32, tag="oev", name=f"oev{h}_{qb}")
                    nc.vector.tensor_copy(oev[:], pv[:])
                    # transpose back to [q, d], normalize rows by the denom
                    otr = ps1pool.tile(
                        [P, 4, D + 1], F32, tag="tr", bufs=tr_bufs, name=f"otr{h}_{qb}"
                    )
                    rcp = wpool.tile([P, 4], F32, tag="rcp", name=f"rcp{h}_{qb}")
                    oout = wpool.tile([P, 4, D], F32, tag="oout", name=f"oout{h}_{qb}")
                    for t in range(4):
                        nc.tensor.transpose(
                            otr[:, t, :],
                            oev[:, P * t : P * (t + 1)],
                            identity[0 : D + 1, 0 : D + 1],
                        )
                        nc.vector.reciprocal(rcp[:, t : t + 1], otr[:, t, D : D + 1])
                        nc.vector.tensor_scalar(
                            oout[:, t, :],
                            otr[:, t, 0:D],
                            rcp[:, t : t + 1],
                            None,
                            mybir.AluOpType.mult,
                        )
                    nc.sync.dma_start(
                        o[h].rearrange("(p c) d -> p c d", p=P)[
                            :, 4 * qb : 4 * (qb + 1), :
                        ],
                        oout[:],
                    )
                    if qb == 0 and h + 1 < HPC:
                        q_nat_next = load_q(h + 1)
                        prep_qT(h + 1, q_nat_next)
    nc.compile()
    return nc


def _get_module(reps=1, **cfg):
    key = (reps, tuple(sorted(cfg.items())))
    if key not in _CACHED:
        _CACHED[key] = _build_module(reps, **cfg)
    return _CACHED[key]


def make_in_maps(Q, K, V):
    """Shard full inputs into per-core input maps (core c -> batch c//4,
    heads 4*(c%4)..4*(c%4)+4)."""
    Q = np.asarray(Q, dtype=np.float32)
    K = np.asarray(K, dtype=np.float32)
    V = np.asarray(V, dtype=np.float32)
    in_maps = []
    for c in range(N_CORES):
        b = c // (N_CORES // B)
        h0 = HPC * (c % (N_CORES // B))
        in_maps.append(
            {
                "q": np.ascontiguousarray(Q[b, h0 : h0 + HPC]),
                "k": np.ascontiguousarray(K[b, 0]),
                "v": np.ascontiguousarray(V[b, 0]),
            }
        )
    return in_maps


def assemble_output(results):
    out = np.empty((B, H, S, D), dtype=np.float32)
    for c in range(N_CORES):
        b = c // (N_CORES // B)
        h0 = HPC * (c % (N_CORES // B))
        out[b, h0 : h0 + HPC] = results[c]["o"]
    return out


def kernel(Q, K, V):
    nc = _get_module(1, **DEFAULT_CFG)
    res = run_bass_kernel_spmd(nc, make_in_maps(Q, K, V), core_ids=list(range(N_CORES)))
    return assemble_output(res.results)



# revision 5
# speedup vs baseline: 1.3352x; 1.3352x over previous
"""MQA attention kernel for Trainium2 (8 NeuronCores, Bass/Tile).

Problem: Q [2,16,2048,64], K/V [2,1,2048,64] fp32, out = softmax(QK^T/8) V.

Sharding: 32 (batch, head) pairs over 8 cores -> 4 heads per core; each core
gets one batch's K/V (replicated across the 4 cores of that batch).

Per-core algorithm (S^T orientation so softmax reduction lands on the free dim
and PV needs no transposition of P):
  - K^T, Q^T built on-chip via PE transposes (d=64 on partitions, zero-padded
    to 128 so all matmuls contract over the full partition dim).
  - S^T[j, q] = (K Q^T) computed in fp32r matmuls (1 cyc/row at free>=256),
    PSUM bank groups per j-chunk.
  - Score evacuation (exp(s/8) fused with PSUM->SBUF) is SPLIT across three
    engines per the `sched` config: ACT runs the real exp; DVE and Pool
    (gpsimd) run a Schraudolph-style approximate exp -- one tensor_scalar
    computing int32(round(A*s + B)) whose bit pattern IS exp(s*scale) to
    ~+/-3% (fp32 bit trick).  Splitting keeps the PE fed so it ramps to and
    stays at the 2.4 GHz p-state instead of throttling at 1.2 GHz.
  - PV uses V augmented with a ones column: one matmul chain yields both
    O^T = V'^T P^T and the softmax denominators (row 64).
  - Software pipelining: stage i's QK matmuls are emitted BEFORE stage i-1's
    PV matmuls (pT is triple-buffered), so evacuation engines always have
    score banks to drain while the PE streams PV.
  - O'^T transposed back with PE, normalized with DVE reciprocal+mul, DMA out.

The q-rows are processed in an interleaved order (partition p holds rows
16p+c) so every DMA moves contiguous 4KB runs; the same rearrange on the
output store undoes the permutation.
"""

import math

import numpy as np

import concourse.bass as bass
import concourse.mybir as mybir
import concourse.tile as tile
from concourse import bacc
from concourse.bass_utils import run_bass_kernel_spmd
from concourse.masks import make_identity

B, H, S, D = 2, 16, 2048, 64
N_CORES = 8
HPC = (B * H) // N_CORES  # heads per core = 4
P = 128
NJ = S // P               # 16 key chunks of 128
QB = 512                  # queries per block (= max fp32 matmul free dim)
NQB = S // QB             # 4 q-blocks per head
SCALE = 1.0 / float(D) ** 0.5
F32 = mybir.dt.float32
F32R = mybir.dt.float32r  # 4-byte matmul dtype, 1 cyc/row at free dim >= 256
BF16 = mybir.dt.bfloat16
I16 = mybir.dt.int16

# Schraudolph exp constants in bf16 (scale folded into A):
#   exp(s*SCALE) ~= bitcast_bf16(int16(A_SCH * s + B_SCH)), max rel err ~3%.
A_SCH = float(np.float32(2.0**7 / math.log(2.0) * SCALE))
B_SCH = float(np.float32(127 * 2**7 - 7.5))

# Per-qb score-evacuation schedule: tuple of (engine, chunk_count) covering
# the NJ=16 j-chunks in order. 'a' = ACT exp (exact), 'd' = DVE schraudolph.
# (GpSimd/Pool cannot access PSUM, so it cannot help evacuate scores.)
DEFAULT_SCHED = ("d1", "a2", "d1", "a2", "d1", "a2", "d1", "a2", "d1", "a2", "d1")

_CACHED = {}
# Best measured config (profiled on HW); kernel() uses this.
DEFAULT_CFG = {}


def _build_module(reps=1, **cfg):
    nc = bacc.Bacc(None)
    q = nc.dram_tensor("q", [HPC, S, D], F32, kind="ExternalInput")
    k = nc.dram_tensor("k", [S, D], F32, kind="ExternalInput")
    v = nc.dram_tensor("v", [S, D], F32, kind="ExternalInput")
    o = nc.dram_tensor("o", [HPC, S, D], F32, kind="ExternalOutput")

    with tile.TileContext(nc) as tc:
        with tc.tile_pool(name="const", bufs=1) as cpool:
            identity = cpool.tile([P, P], F32)
            make_identity(nc, identity)

            kT = cpool.tile([P, S], F32R)
            nc.gpsimd.memset(kT[64:P, :].bitcast(mybir.dt.uint32), 0)
            vp = cpool.tile([P, NJ, D + 1], BF16)
            nc.gpsimd.memset(vp[:, :, D], 1.0)
            qT_tiles = []
            for i in range(2):
                qTt = cpool.tile([P, S], F32R, name=f"qT{i}")
                nc.gpsimd.memset(qTt[64:P, :].bitcast(mybir.dt.uint32), 0)
                qT_tiles.append(qTt)

            for rep in range(reps):
                _trace_body(nc, tc, q, k, v, o, identity, kT, vp, qT_tiles, **cfg)
    nc.compile()
    return nc


def _trace_body(
    nc, tc, q, k, v, o, identity, kT, vp, qT_tiles,
    sched=None, act_bufs=2, s1_bufs=2, pt_bufs=3, tr_bufs=1, pv_bufs=1,
    cast_eng="v", oev_eng="v",
):
    sched = sched if sched is not None else DEFAULT_SCHED
    groups = [(g[0], int(g[1:])) for g in sched]
    assert sum(gsz for _, gsz in groups) == NJ
    act_pad = max([gsz for eng, gsz in groups if eng == "a"] or [1])
    s1_pad = max([gsz for eng, gsz in groups if eng != "a"] or [1])
    cast_engine = {"g": nc.gpsimd, "v": nc.vector}[cast_eng]
    oev_engine = {"g": nc.gpsimd, "v": nc.vector}[oev_eng]

    with (
        tc.tile_pool(name="natb", bufs=2) as npool,
        tc.tile_pool(name="workb", bufs=2) as wpool,
        tc.tile_pool(name="ptb", bufs=pt_bufs) as ptpool,
        tc.tile_pool(name="psab", bufs=act_bufs, space="PSUM") as psa,
        tc.tile_pool(name="pssb", bufs=s1_bufs, space="PSUM") as pss,
        tc.tile_pool(name="ps1b", bufs=1, space="PSUM") as ps1,
    ):
            def transpose_64(dst, src_nat, who):
                # PE-transpose 4 [128,64] chunks into one PSUM tile, then one
                # copy into [64, 512] of the fp32r destination.
                for g in range(NJ // 4):
                    pst = ps1.tile(
                        [64, 4, P], F32, tag="tr", bufs=tr_bufs, name=f"pst_{who}{g}"
                    )
                    for t in range(4):
                        nc.tensor.transpose(
                            pst[:, t, :], src_nat[:, 4 * g + t, :], identity
                        )
                    cast_engine.tensor_copy(dst[0:64, 512 * g : 512 * (g + 1)], pst[:])

            def load_q(h):
                q_nat = npool.tile([P, NJ, D], F32, tag="nat", name=f"q_nat{h}")
                nc.sync.dma_start(q_nat[:], q[h].rearrange("(p c) d -> p c d", p=P))
                return q_nat

            # ---- startup: K^T and head-0 Q^T, transposed interleaved ----
            k_nat = npool.tile([P, NJ, D], F32, tag="nat")
            nc.sync.dma_start(k_nat[:], k.rearrange("(p c) d -> p c d", p=P))
            q_nat_next = load_q(0)
            for g in range(NJ // 4):
                pstk = ps1.tile([64, 4, P], F32, tag="tr", bufs=tr_bufs, name=f"pst_k{g}")
                for t in range(4):
                    nc.tensor.transpose(pstk[:, t, :], k_nat[:, 4 * g + t, :], identity)
                cast_engine.tensor_copy(kT[0:64, 512 * g : 512 * (g + 1)], pstk[:])
                pstq = ps1.tile([64, 4, P], F32, tag="tr", bufs=tr_bufs, name=f"pst_q0{g}")
                for t in range(4):
                    nc.tensor.transpose(
                        pstq[:, t, :], q_nat_next[:, 4 * g + t, :], identity
                    )
                cast_engine.tensor_copy(
                    qT_tiles[0][0:64, 512 * g : 512 * (g + 1)], pstq[:]
                )

            # ---- V' [128, 16, 65]: V plus a ones column (softmax denom) ----
            v_nat = npool.tile([P, NJ, D], F32, tag="nat", name="v_nat")
            nc.sync.dma_start(v_nat[:], v.rearrange("(p c) d -> p c d", p=P))
            nc.vector.tensor_copy(vp[:, :, 0:D], v_nat[:])

            def emit_qk(idx, h, qb):
                """QK^T matmuls for one 512-query block + 3-engine evacuation."""
                qT = qT_tiles[h % 2]
                qs = qT[:, QB * qb : QB * (qb + 1)]
                pT = ptpool.tile([P, NJ * QB], BF16, tag="pT", name=f"pT{idx % pt_bufs}")
                j0 = 0
                for gi, (eng, gsz) in enumerate(groups):
                    if eng == "a":
                        sg = psa.tile(
                            [P, gsz, QB], F32, tag="sga",
                            name=f"sga{idx}_{gi}", padded_shape=[P, act_pad, QB],
                        )
                    else:
                        sg = pss.tile(
                            [P, gsz, QB], F32, tag="sgs",
                            name=f"sgs{idx}_{gi}", padded_shape=[P, s1_pad, QB],
                        )
                    for i in range(gsz):
                        j = j0 + i
                        nc.tensor.matmul(
                            sg[:, i, :],
                            lhsT=kT[:, P * j : P * (j + 1)],
                            rhs=qs,
                            start=True,
                            stop=True,
                        )
                    dst = pT[:, QB * j0 : QB * (j0 + gsz)]
                    if eng == "a":
                        nc.scalar.activation(
                            dst, sg[:], mybir.ActivationFunctionType.Exp, scale=SCALE
                        )
                    else:
                        eng_obj = nc.vector if eng == "d" else nc.gpsimd
                        eng_obj.tensor_scalar(
                            dst.bitcast(I16), sg[:], A_SCH, B_SCH,
                            mybir.AluOpType.mult, mybir.AluOpType.add,
                        )
                    j0 += gsz
                return pT

            def emit_pv_out(idx, h, qb, pT):
                """PV accumulation, transpose back, normalize, store."""
                pv = ps1.tile([D + 1, QB], F32, tag="pv", bufs=pv_bufs, name=f"pv{idx}")
                for c in range(NJ):
                    nc.tensor.matmul(
                        pv[:],
                        lhsT=vp[:, c, :],
                        rhs=pT[:, QB * c : QB * (c + 1)],
                        start=(c == 0),
                        stop=(c == NJ - 1),
                    )
                oev = wpool.tile([D + 1, QB], F32, tag="oev", name=f"oev{idx}")
                oev_engine.tensor_copy(oev[:], pv[:])
                otr = ps1.tile(
                    [P, 4, D + 1], F32, tag="tr", bufs=tr_bufs, name=f"otr{idx}"
                )
                rcp = wpool.tile([P, 4], F32, tag="rcp", name=f"rcp{idx}")
                oout = wpool.tile([P, 4, D], F32, tag="oout", name=f"oout{idx}")
                for t in range(4):
                    nc.tensor.transpose(
                        otr[:, t, :],
                        oev[:, P * t : P * (t + 1)],
                        identity[0 : D + 1, 0 : D + 1],
                    )
                nc.vector.reciprocal(rcp[:], otr[:, :, D : D + 1])
                for t in range(4):
                    nc.vector.tensor_scalar(
                        oout[:, t, :],
                        otr[:, t, 0:D],
                        rcp[:, t : t + 1],
                        None,
                        mybir.AluOpType.mult,
                    )
                nc.sync.dma_start(
                    o[h].rearrange("(p c) d -> p c d", p=P)[
                        :, 4 * qb : 4 * (qb + 1), :
                    ],
                    oout[:],
                )

            stages = [(h, qb) for h in range(HPC) for qb in range(NQB)]
            prev = None
            for idx, (h, qb) in enumerate(stages):
                pT = emit_qk(idx, h, qb)
                if prev is not None:
                    emit_pv_out(*prev)
                if qb == 0 and h + 1 < HPC:
                    q_nat_next = load_q(h + 1)
                    transpose_64(qT_tiles[(h + 1) % 2], q_nat_next, f"q{h + 1}_")
                prev = (idx, h, qb, pT)
            emit_pv_out(*prev)
    nc.compile()
    return nc


def _get_module(reps=1, **cfg):
    key = (reps, tuple(sorted((k, tuple(v) if isinstance(v, (list, tuple)) else v)
                              for k, v in cfg.items())))
    if key not in _CACHED:
        _CACHED[key] = _build_module(reps, **cfg)
    return _CACHED[key]


def make_in_maps(Q, K, V):
    """Shard full inputs into per-core input maps (core c -> batch c//4,
    heads 4*(c%4)..4*(c%4)+4)."""
    Q = np.asarray(Q, dtype=np.float32)
    K = np.asarray(K, dtype=np.float32)
    V = np.asarray(V, dtype=np.float32)
    in_maps = []
    for c in range(N_CORES):
        b = c // (N_CORES // B)
        h0 = HPC * (c % (N_CORES // B))
        in_maps.append(
            {
                "q": np.ascontiguousarray(Q[b, h0 : h0 + HPC]),
                "k": np.ascontiguousarray(K[b, 0]),
                "v": np.ascontiguousarray(V[b, 0]),
            }
        )
    return in_maps


def assemble_output(results):
    out = np.empty((B, H, S, D), dtype=np.float32)
    for c in range(N_CORES):
        b = c // (N_CORES // B)
        h0 = HPC * (c % (N_CORES // B))
        out[b, h0 : h0 + HPC] = results[c]["o"]
    return out


def kernel(Q, K, V):
    nc = _get_module(1, **DEFAULT_CFG)
    res = run_bass_kernel_spmd(nc, make_in_maps(Q, K, V), core_ids=list(range(N_CORES)))
    return assemble_output(res.results)


# revision 8
# speedup vs baseline: 1.3931x; 1.0433x over previous
"""MQA attention kernel for Trainium2 (8 NeuronCores, Bass/Tile).

Problem: Q [2,16,2048,64], K/V [2,1,2048,64] fp32, out = softmax(QK^T/8) V.

Sharding: 32 (batch, head) pairs over 8 cores -> 4 heads per core; each core
gets one batch's K/V (replicated across the 4 cores of that batch).

Per-core algorithm (S^T orientation so softmax reduction lands on the free dim
and PV needs no transposition of P):
  - K^T, Q^T built on-chip via PE transposes (d=64 on partitions, zero-padded
    to 128 so all matmuls contract over the full partition dim).
  - S^T[j, q] = (K Q^T) computed in fp32r matmuls (1 cyc/row at free>=256),
    PSUM bank groups per j-chunk.
  - Score evacuation (exp(s/8) fused with PSUM->SBUF) is SPLIT across three
    engines per the `sched` config: ACT runs the real exp; DVE and Pool
    (gpsimd) run a Schraudolph-style approximate exp -- one tensor_scalar
    computing int32(round(A*s + B)) whose bit pattern IS exp(s*scale) to
    ~+/-3% (fp32 bit trick).  Splitting keeps the PE fed so it ramps to and
    stays at the 2.4 GHz p-state instead of throttling at 1.2 GHz.
  - PV uses V augmented with a ones column: one matmul chain yields both
    O^T = V'^T P^T and the softmax denominators (row 64).
  - Software pipelining: stage i's QK matmuls are emitted BEFORE stage i-1's
    PV matmuls (pT is triple-buffered), so evacuation engines always have
    score banks to drain while the PE streams PV.
  - O'^T transposed back with PE, normalized with DVE reciprocal+mul, DMA out.

The q-rows are processed in an interleaved order (partition p holds rows
16p+c) so every DMA moves contiguous 4KB runs; the same rearrange on the
output store undoes the permutation.
"""

import math

import numpy as np

import concourse.bass as bass
import concourse.mybir as mybir
import concourse.tile as tile
from concourse import bacc
from concourse.bass_utils import run_bass_kernel_spmd
from concourse.masks import make_identity

B, H, S, D = 2, 16, 2048, 64
N_CORES = 8
HPC = (B * H) // N_CORES  # heads per core = 4
P = 128
NJ = S // P               # 16 key chunks of 128
QB = 512                  # queries per block (= max fp32 matmul free dim)
NQB = S // QB             # 4 q-blocks per head
SCALE = 1.0 / float(D) ** 0.5
F32 = mybir.dt.float32
F32R = mybir.dt.float32r  # 4-byte matmul dtype, 1 cyc/row at free dim >= 256
BF16 = mybir.dt.bfloat16
I16 = mybir.dt.int16

# Schraudolph exp constants in bf16 (scale folded into A):
#   exp(s*SCALE) ~= bitcast_bf16(int16(A_SCH * s + B_SCH)), max rel err ~3%.
A_SCH = float(np.float32(2.0**7 / math.log(2.0) * SCALE))
B_SCH = float(np.float32(127 * 2**7 - 7.5))

# Per-qb score-evacuation schedule: tuple of (engine, chunk_count) covering
# the NJ=16 j-chunks in order. 'a' = ACT exp (exact), 'd' = DVE schraudolph.
# (GpSimd/Pool cannot access PSUM, so it cannot help evacuate scores.)
DEFAULT_SCHED = ("d1", "a2", "d1", "a2", "d1", "a2", "d1", "a2", "d1", "a2", "d1")

_CACHED = {}
# Best measured config (profiled on HW); kernel() uses this.
DEFAULT_CFG = {}


def _build_module(reps=1, **cfg):
    nc = bacc.Bacc(None)
    q = nc.dram_tensor("q", [HPC, S, D], F32, kind="ExternalInput")
    k = nc.dram_tensor("k", [S, D], F32, kind="ExternalInput")
    v = nc.dram_tensor("v", [S, D], F32, kind="ExternalInput")
    o = nc.dram_tensor("o", [HPC, S, D], F32, kind="ExternalOutput")

    with tile.TileContext(nc) as tc:
        with tc.tile_pool(name="const", bufs=1) as cpool:
            identity = cpool.tile([P, P], F32)
            make_identity(nc, identity)

            kT = cpool.tile([P, S], BF16)
            nc.gpsimd.memset(kT[64:P, :], 0.0)
            vp = cpool.tile([P, NJ, D + 1], BF16)
            nc.gpsimd.memset(vp[:, :, D], 1.0)
            qT_tiles = []
            for i in range(2):
                qTt = cpool.tile([P, S], BF16, name=f"qT{i}")
                nc.gpsimd.memset(qTt[64:P, :], 0.0)
                qT_tiles.append(qTt)

            for rep in range(reps):
                _trace_body(nc, tc, q, k, v, o, identity, kT, vp, qT_tiles, **cfg)
    nc.compile()
    return nc


def _trace_body(
    nc, tc, q, k, v, o, identity, kT, vp, qT_tiles,
    sched=None, act_bufs=2, s1_bufs=2, pt_bufs=3, tr_bufs=1, pv_bufs=1,
    cast_eng="v", oev_eng="v", norm_eng="v",
):
    sched = sched if sched is not None else DEFAULT_SCHED
    groups = [(g[0], int(g[1:])) for g in sched]
    assert sum(gsz for _, gsz in groups) == NJ
    act_pad = max([gsz for eng, gsz in groups if eng == "a"] or [1])
    s1_pad = max([gsz for eng, gsz in groups if eng != "a"] or [1])
    cast_engine = {"g": nc.gpsimd, "v": nc.vector}[cast_eng]
    oev_engine = {"g": nc.gpsimd, "v": nc.vector}[oev_eng]

    with (
        tc.tile_pool(name="natb", bufs=2) as npool,
        tc.tile_pool(name="workb", bufs=2) as wpool,
        tc.tile_pool(name="ptb", bufs=pt_bufs) as ptpool,
        tc.tile_pool(name="psab", bufs=act_bufs, space="PSUM") as psa,
        tc.tile_pool(name="pssb", bufs=s1_bufs, space="PSUM") as pss,
        tc.tile_pool(name="ps1b", bufs=1, space="PSUM") as ps1,
    ):
            def transpose_64(dst, src_nat, who):
                # PE-transpose 4 [128,64] chunks into one PSUM tile, then one
                # casting copy into [64, 512] of the bf16 destination.
                for g in range(NJ // 4):
                    pst = ps1.tile(
                        [64, 4, P], F32, tag="tr", bufs=tr_bufs, name=f"pst_{who}{g}"
                    )
                    for t in range(4):
                        nc.tensor.transpose(
                            pst[:, t, :], src_nat[:, 4 * g + t, :], identity
                        )
                    cast_engine.tensor_copy(dst[0:64, 512 * g : 512 * (g + 1)], pst[:])

            def load_q(h, split=1):
                q_nat = npool.tile([P, NJ, D], F32, tag="nat", name=f"q_nat{h}")
                qsrc = q[h].rearrange("(p c) d -> p c d", p=P)
                cs = NJ // split
                for i in range(split):
                    nc.sync.dma_start(
                        q_nat[:, cs * i : cs * (i + 1), :],
                        qsrc[:, cs * i : cs * (i + 1), :],
                    )
                return q_nat

            # ---- startup: K^T and head-0 Q^T, transposed interleaved ----
            # (K/Q0 loads split into 4 so transposes start after the first.)
            k_nat = npool.tile([P, NJ, D], F32, tag="nat")
            ksrc = k.rearrange("(p c) d -> p c d", p=P)
            for i in range(4):
                nc.sync.dma_start(
                    k_nat[:, 4 * i : 4 * (i + 1), :], ksrc[:, 4 * i : 4 * (i + 1), :]
                )
            q_nat_next = load_q(0, split=4)
            for g in range(NJ // 4):
                pstk = ps1.tile([64, 4, P], F32, tag="tr", bufs=tr_bufs, name=f"pst_k{g}")
                for t in range(4):
                    nc.tensor.transpose(pstk[:, t, :], k_nat[:, 4 * g + t, :], identity)
                cast_engine.tensor_copy(kT[0:64, 512 * g : 512 * (g + 1)], pstk[:])
                pstq = ps1.tile([64, 4, P], F32, tag="tr", bufs=tr_bufs, name=f"pst_q0{g}")
                for t in range(4):
                    nc.tensor.transpose(
                        pstq[:, t, :], q_nat_next[:, 4 * g + t, :], identity
                    )
                cast_engine.tensor_copy(
                    qT_tiles[0][0:64, 512 * g : 512 * (g + 1)], pstq[:]
                )

            # ---- V' [128, 16, 65]: V plus a ones column (softmax denom) ----
            v_nat = npool.tile([P, NJ, D], F32, tag="nat", name="v_nat")
            nc.sync.dma_start(v_nat[:], v.rearrange("(p c) d -> p c d", p=P))
            nc.vector.tensor_copy(vp[:, :, 0:D], v_nat[:])

            def emit_qk(idx, h, qb):
                """QK^T matmuls for one 512-query block + 3-engine evacuation."""
                qT = qT_tiles[h % 2]
                qs = qT[:, QB * qb : QB * (qb + 1)]
                pT = ptpool.tile([P, NJ * QB], BF16, tag="pT", name=f"pT{idx % pt_bufs}")
                j0 = 0
                for gi, (eng, gsz) in enumerate(groups):
                    if eng == "a":
                        sg = psa.tile(
                            [P, gsz, QB], F32, tag="sga",
                            name=f"sga{idx}_{gi}", padded_shape=[P, act_pad, QB],
                        )
                    else:
                        sg = pss.tile(
                            [P, gsz, QB], F32, tag="sgs",
                            name=f"sgs{idx}_{gi}", padded_shape=[P, s1_pad, QB],
                        )
                    for i in range(gsz):
                        j = j0 + i
                        nc.tensor.matmul(
                            sg[:, i, :],
                            lhsT=kT[:, P * j : P * (j + 1)],
                            rhs=qs,
                            start=True,
                            stop=True,
                        )
                    dst = pT[:, QB * j0 : QB * (j0 + gsz)]
                    if eng == "a":
                        nc.scalar.activation(
                            dst, sg[:], mybir.ActivationFunctionType.Exp, scale=SCALE
                        )
                    else:
                        eng_obj = nc.vector if eng == "d" else nc.gpsimd
                        eng_obj.tensor_scalar(
                            dst.bitcast(I16), sg[:], A_SCH, B_SCH,
                            mybir.AluOpType.mult, mybir.AluOpType.add,
                        )
                    j0 += gsz
                return pT

            def emit_pv_out(idx, h, qb, pT):
                """PV accumulation, transpose back, normalize, store."""
                pv = ps1.tile([D + 1, QB], F32, tag="pv", bufs=pv_bufs, name=f"pv{idx}")
                for c in range(NJ):
                    nc.tensor.matmul(
                        pv[:],
                        lhsT=vp[:, c, :],
                        rhs=pT[:, QB * c : QB * (c + 1)],
                        start=(c == 0),
                        stop=(c == NJ - 1),
                    )
                oev = wpool.tile([D + 1, QB], F32, tag="oev", name=f"oev{idx}")
                oev_engine.tensor_copy(oev[:], pv[:])
                otr = ps1.tile(
                    [P, 4, D + 1], F32, tag="tr", bufs=tr_bufs, name=f"otr{idx}"
                )
                rcp = wpool.tile([P, 4], F32, tag="rcp", name=f"rcp{idx}")
                oout = wpool.tile([P, 4, D], F32, tag="oout", name=f"oout{idx}")
                for t in range(4):
                    nc.tensor.transpose(
                        otr[:, t, :],
                        oev[:, P * t : P * (t + 1)],
                        identity[0 : D + 1, 0 : D + 1],
                    )
                nc.vector.reciprocal(rcp[:], otr[:, :, D : D + 1])
                for t in range(4):
                    if norm_eng == "a":
                        nc.scalar.activation(
                            oout[:, t, :],
                            otr[:, t, 0:D],
                            mybir.ActivationFunctionType.Copy,
                            scale=rcp[:, t : t + 1],
                        )
                    else:
                        nc.vector.tensor_scalar(
                            oout[:, t, :],
                            otr[:, t, 0:D],
                            rcp[:, t : t + 1],
                            None,
                            mybir.AluOpType.mult,
                        )
                nc.sync.dma_start(
                    o[h].rearrange("(p c) d -> p c d", p=P)[
                        :, 4 * qb : 4 * (qb + 1), :
                    ],
                    oout[:],
                )

            stages = [(h, qb) for h in range(HPC) for qb in range(NQB)]
            prev = None
            for idx, (h, qb) in enumerate(stages):
                pT = emit_qk(idx, h, qb)
                if prev is not None:
                    emit_pv_out(*prev)
                if qb == 0 and h + 1 < HPC:
                    q_nat_next = load_q(h + 1)
                    transpose_64(qT_tiles[(h + 1) % 2], q_nat_next, f"q{h + 1}_")
                prev = (idx, h, qb, pT)
            emit_pv_out(*prev)
    nc.compile()
    return nc


def _get_module(reps=1, **cfg):
    key = (reps, tuple(sorted((k, tuple(v) if isinstance(v, (list, tuple)) else v)
                              for k, v in cfg.items())))
    if key not in _CACHED:
        _CACHED[key] = _build_module(reps, **cfg)
    return _CACHED[key]


def make_in_maps(Q, K, V):
    """Shard full inputs into per-core input maps (core c -> batch c//4,
    heads 4*(c%4)..4*(c%4)+4)."""
    Q = np.asarray(Q, dtype=np.float32)
    K = np.asarray(K, dtype=np.float32)
    V = np.asarray(V, dtype=np.float32)
    in_maps = []
    for c in range(N_CORES):
        b = c // (N_CORES // B)
        h0 = HPC * (c % (N_CORES // B))
        in_maps.append(
            {
                "q": np.ascontiguousarray(Q[b, h0 : h0 + HPC]),
                "k": np.ascontiguousarray(K[b, 0]),
                "v": np.ascontiguousarray(V[b, 0]),
            }
        )
    return in_maps


def assemble_output(results):
    out = np.empty((B, H, S, D), dtype=np.float32)
    for c in range(N_CORES):
        b = c // (N_CORES // B)
        h0 = HPC * (c % (N_CORES // B))
        out[b, h0 : h0 + HPC] = results[c]["o"]
    return out


def kernel(Q, K, V):
    nc = _get_module(1, **DEFAULT_CFG)
    res = run_bass_kernel_spmd(nc, make_in_maps(Q, K, V), core_ids=list(range(N_CORES)))
    return assemble_output(res.results)


# revision 19
# speedup vs baseline: 1.4180x; 1.0179x over previous
"""MQA attention kernel for Trainium2 (8 NeuronCores, Bass/Tile).

Problem: Q [2,16,2048,64], K/V [2,1,2048,64] fp32, out = softmax(QK^T/8) V.

Sharding: 32 (batch, head) pairs over 8 cores -> 4 heads per core; each core
gets one batch's K/V (replicated across the 4 cores of that batch).

Per-core algorithm (S^T orientation so softmax reduction lands on the free dim
and PV needs no transposition of P):
  - Q/K are cast to bf16 on the Pool engine (idle otherwise), transposed with
    PE transpose-mode into K^T/Q^T [128, S] bf16 (d=64 zero-padded to 128;
    64-row stationaries measured slower, likely losing fast weight load).
  - S^T[j, q] = (K Q^T) in bf16 matmuls (1 cyc/row, FWL weight loads),
    one PSUM bank group per 1-2 j-chunks.
  - Score evacuation (exp(s/8) fused with PSUM->SBUF) is SPLIT between the
    ACT engine (real exp) and the DVE (Schraudolph approximate exp: one
    tensor_scalar computing int16(A*s + B) whose bit pattern IS bf16
    exp(s*scale) to ~+/-3%).  The split keeps the PE continuously fed so it
    stays at the 2.4 GHz p-state.  (GpSimd cannot access PSUM.)
  - PV uses bf16 V augmented with a ones column: one matmul chain yields both
    O^T = V'^T P^T and the softmax denominators (row 64).
  - Software pipelining: stage i's QK matmuls are emitted BEFORE stage i-1's
    PV matmuls (pT is triple-buffered), so evacuation engines always have
    score banks to drain while the PE streams PV.
  - O'^T evacuated to bf16, transposed back with PE, normalized with DVE
    reciprocal + per-partition multiply into fp32, DMA out.

The q-rows are processed in an interleaved order (partition p holds rows
16p+c) so every DMA moves contiguous 4KB runs; the same rearrange on the
output store undoes the permutation.
"""

import math

import numpy as np

import concourse.mybir as mybir
import concourse.tile as tile
from concourse import bacc
from concourse.bass_utils import run_bass_kernel_spmd
from concourse.masks import make_identity

B, H, S, D = 2, 16, 2048, 64
N_CORES = 8
HPC = (B * H) // N_CORES  # heads per core = 4
P = 128
NJ = S // P               # 16 key chunks of 128
QB = 512                  # queries per block (= one PSUM bank of fp32 scores)
NQB = S // QB             # 4 q-blocks per head
SCALE = 1.0 / float(D) ** 0.5
F32 = mybir.dt.float32
BF16 = mybir.dt.bfloat16
I16 = mybir.dt.int16

# Schraudolph exp constants in bf16 (scale folded into A):
#   exp(s*SCALE) ~= bitcast_bf16(int16(A_SCH * s + B_SCH)), max rel err ~3%.
A_SCH = float(np.float32(2.0**7 / math.log(2.0) * SCALE))
B_SCH = float(np.float32(127 * 2**7 - 7.5))

# Per-qb score-evacuation schedule: tuple of (engine, chunk_count) covering
# the NJ=16 j-chunks in order. 'a' = ACT exp (exact), 'd' = DVE schraudolph.
DEFAULT_SCHED = ("d1", "a2", "d1", "a2", "d1", "a2", "d1", "a2", "d1", "a2", "d1")

_CACHED = {}
# Best measured config (profiled on HW); kernel() uses this.
DEFAULT_CFG = {}


def _build_module(reps=1, **cfg):
    nc = bacc.Bacc(None)
    q = nc.dram_tensor("q", [HPC, S, D], F32, kind="ExternalInput")
    k = nc.dram_tensor("k", [S, D], F32, kind="ExternalInput")
    v = nc.dram_tensor("v", [S, D], F32, kind="ExternalInput")
    o = nc.dram_tensor("o", [HPC, S, D], F32, kind="ExternalOutput")

    with tile.TileContext(nc) as tc:
        with tc.tile_pool(name="const", bufs=1) as cpool:
            identity = cpool.tile([P, P], BF16)
            make_identity(nc, identity)

            kT = cpool.tile([P, S], BF16)
            nc.gpsimd.memset(kT[64:P, :], 0.0)
            vp = cpool.tile([P, NJ, D + 1], BF16)
            nc.gpsimd.memset(vp[:, :, D], 1.0)
            qT_tiles = []
            for i in range(2):
                qTt = cpool.tile([P, S], BF16, name=f"qT{i}")
                nc.gpsimd.memset(qTt[64:P, :], 0.0)
                qT_tiles.append(qTt)

            for rep in range(reps):
                _trace_body(nc, tc, q, k, v, o, identity, kT, vp, qT_tiles, **cfg)
    nc.compile()
    return nc


def _trace_body(
    nc, tc, q, k, v, o, identity, kT, vp, qT_tiles,
    sched=None, act_bufs=2, s1_bufs=2, pt_bufs=3, tr_bufs=1, pv_bufs=1,
    norm_eng="v", qkT_dma=False, otr_dma=False,
):
    sched = sched if sched is not None else DEFAULT_SCHED
    groups = [(g[0], int(g[1:])) for g in sched]
    assert sum(gsz for _, gsz in groups) == NJ
    act_pad = max([gsz for eng, gsz in groups if eng == "a"] or [1])
    s1_pad = max([gsz for eng, gsz in groups if eng != "a"] or [1])

    with (
        tc.tile_pool(name="natb", bufs=2) as npool,
        tc.tile_pool(name="workb", bufs=2) as wpool,
        tc.tile_pool(name="ptb", bufs=pt_bufs) as ptpool,
        tc.tile_pool(name="psab", bufs=act_bufs, space="PSUM") as psa,
        tc.tile_pool(name="pssb", bufs=s1_bufs, space="PSUM") as pss,
        tc.tile_pool(name="ps1b", bufs=1, space="PSUM") as ps1,
    ):
            def cast_bf(h, src_nat, split=1):
                """Pool-engine fp32 -> bf16 cast (SBUF->SBUF; Pool is idle)."""
                bf = npool.tile([P, NJ, D], BF16, tag="bf", name=f"bf{h}")
                cs = NJ // split
                for i in range(split):
                    nc.gpsimd.tensor_copy(
                        bf[:, cs * i : cs * (i + 1), :],
                        src_nat[:, cs * i : cs * (i + 1), :],
                    )
                return bf

            def transpose_64_dma(dst, src_bf):
                # XBAR DMA transpose: [128, 64] bf16 chunks -> [64, 128] of
                # the destination, dispatched from the SP queue (no PE/DVE).
                for c in range(NJ):
                    nc.sync.dma_start_transpose(
                        dst[:, P * c : P * (c + 1)], src_bf[:, c, :]
                    )

            def transpose_64(dst, src_bf, who):
                # PE-transpose 4 [128,64] bf16 chunks into one PSUM tile, then
                # one DVE copy into [64, 512] of the bf16 destination.
                for g in range(NJ // 4):
                    pst = ps1.tile(
                        [64, 4, P], BF16, tag="tr", bufs=tr_bufs, name=f"pst_{who}{g}"
                    )
                    for t in range(4):
                        nc.tensor.transpose(
                            pst[:, t, :], src_bf[:, 4 * g + t, :], identity
                        )
                    nc.vector.tensor_copy(
                        dst[0:64, 512 * g : 512 * (g + 1)], pst[:]
                    )

            def load_q(h, split=1):
                q_nat = npool.tile([P, NJ, D], F32, tag="nat", name=f"q_nat{h}")
                qsrc = q[h].rearrange("(p c) d -> p c d", p=P)
                cs = NJ // split
                for i in range(split):
                    nc.sync.dma_start(
                        q_nat[:, cs * i : cs * (i + 1), :],
                        qsrc[:, cs * i : cs * (i + 1), :],
                    )
                return q_nat

            # ---- startup: K^T and head-0 Q^T, loads/casts/transposes all
            # chunk-interleaved so the first QK matmul can start early ----
            k_nat = npool.tile([P, NJ, D], F32, tag="nat")
            ksrc = k.rearrange("(p c) d -> p c d", p=P)
            for i in range(4):
                nc.sync.dma_start(
                    k_nat[:, 4 * i : 4 * (i + 1), :], ksrc[:, 4 * i : 4 * (i + 1), :]
                )
            q_nat0 = load_q(0, split=4)
            k_bf = cast_bf("k", k_nat, split=4)
            q_bf0 = cast_bf(0, q_nat0, split=4)
            for g in range(NJ // 4):
                pstk = ps1.tile([64, 4, P], BF16, tag="tr", bufs=tr_bufs, name=f"pst_k{g}")
                for t in range(4):
                    nc.tensor.transpose(pstk[:, t, :], k_bf[:, 4 * g + t, :], identity)
                nc.vector.tensor_copy(kT[0:64, 512 * g : 512 * (g + 1)], pstk[:])
                pstq = ps1.tile([64, 4, P], BF16, tag="tr", bufs=tr_bufs, name=f"pst_q0{g}")
                for t in range(4):
                    nc.tensor.transpose(pstq[:, t, :], q_bf0[:, 4 * g + t, :], identity)
                nc.vector.tensor_copy(
                    qT_tiles[0][0:64, 512 * g : 512 * (g + 1)], pstq[:]
                )

            # ---- V' [128, 16, 65] bf16: V plus a ones column ----
            v_nat = npool.tile([P, NJ, D], F32, tag="nat", name="v_nat")
            nc.sync.dma_start(v_nat[:], v.rearrange("(p c) d -> p c d", p=P))
            nc.gpsimd.tensor_copy(vp[:, :, 0:D], v_nat[:])

            def emit_qk(idx, h, qb):
                """QK^T matmuls for one 512-query block + 2-engine evacuation."""
                qT = qT_tiles[h % 2]
                qs = qT[:, QB * qb : QB * (qb + 1)]
                pT = ptpool.tile([P, NJ * QB], BF16, tag="pT", name=f"pT{idx % pt_bufs}")
                j0 = 0
                for gi, (eng, gsz) in enumerate(groups):
                    if eng == "a":
                        sg = psa.tile(
                            [P, gsz, QB], F32, tag="sga",
                            name=f"sga{idx}_{gi}", padded_shape=[P, act_pad, QB],
                        )
                    else:
                        sg = pss.tile(
                            [P, gsz, QB], F32, tag="sgs",
                            name=f"sgs{idx}_{gi}", padded_shape=[P, s1_pad, QB],
                        )
                    for i in range(gsz):
                        j = j0 + i
                        nc.tensor.matmul(
                            sg[:, i, :],
                            lhsT=kT[:, P * j : P * (j + 1)],
                            rhs=qs,
                            start=True,
                            stop=True,
                        )
                    dst = pT[:, QB * j0 : QB * (j0 + gsz)]
                    if eng == "a":
                        nc.scalar.activation(
                            dst, sg[:], mybir.ActivationFunctionType.Exp, scale=SCALE
                        )
                    else:
                        nc.vector.tensor_scalar(
                            dst.bitcast(I16), sg[:], A_SCH, B_SCH,
                            mybir.AluOpType.mult, mybir.AluOpType.add,
                        )
                    j0 += gsz
                return pT

            def emit_pv_out(idx, h, qb, pT):
                """PV accumulation, transpose back, normalize, store."""
                pv = ps1.tile([D + 1, QB], F32, tag="pv", bufs=pv_bufs, name=f"pv{idx}")
                for c in range(NJ):
                    nc.tensor.matmul(
                        pv[:],
                        lhsT=vp[:, c, :],
                        rhs=pT[:, QB * c : QB * (c + 1)],
                        start=(c == 0),
                        stop=(c == NJ - 1),
                    )
                rcp = wpool.tile([P, 4], F32, tag="rcp", name=f"rcp{idx}")
                oout = wpool.tile([P, 4, D], F32, tag="oout", name=f"oout{idx}")
                if otr_dma:
                    # O'^T evacuated into a full-128-partition bf16 tile (rows
                    # 65..127 zeroed for the XBAR), transposed back by the DMA
                    # XBAR in 128x128 blocks -- no PE/PSUM involved, and the
                    # normalize below reads all-SBUF operands (2x DVE mode).
                    oev = wpool.tile([P, QB], BF16, tag="oev", name=f"oev{idx}")
                    nc.vector.memset(oev[D : P, :], 0.0)
                    nc.vector.tensor_copy(oev[0 : D + 1, :], pv[:])
                    otr = wpool.tile([P, 4, P], BF16, tag="otr", name=f"otr{idx}")
                    for t in range(4):
                        nc.sync.dma_start_transpose(
                            otr[:, t, :], oev[:, P * t : P * (t + 1)]
                        )
                else:
                    oev = wpool.tile([D + 1, QB], BF16, tag="oev", name=f"oev{idx}")
                    nc.vector.tensor_copy(oev[:], pv[:])
                    # inner dim padded to D+2 so each bf16 PSUM chunk stays
                    # 4-byte aligned (66*2 = 132B)
                    otr = ps1.tile(
                        [P, 4, D + 2], BF16, tag="tr", bufs=tr_bufs, name=f"otr{idx}"
                    )
                    for t in range(4):
                        nc.tensor.transpose(
                            otr[:, t, 0 : D + 1],
                            oev[:, P * t : P * (t + 1)],
                            identity[0 : D + 1, 0 : D + 1],
                        )
                nc.vector.reciprocal(rcp[:], otr[:, :, D : D + 1])
                for t in range(4):
                    if norm_eng == "a":
                        nc.scalar.activation(
                            oout[:, t, :],
                            otr[:, t, 0:D],
                            mybir.ActivationFunctionType.Copy,
                            scale=rcp[:, t : t + 1],
                        )
                    else:
                        nc.vector.tensor_scalar(
                            oout[:, t, :],
                            otr[:, t, 0:D],
                            rcp[:, t : t + 1],
                            None,
                            mybir.AluOpType.mult,
                        )
                nc.sync.dma_start(
                    o[h].rearrange("(p c) d -> p c d", p=P)[
                        :, 4 * qb : 4 * (qb + 1), :
                    ],
                    oout[:],
                )

            stages = [(h, qb) for h in range(HPC) for qb in range(NQB)]
            prev = None
            for idx, (h, qb) in enumerate(stages):
                pT = emit_qk(idx, h, qb)
                if prev is not None:
                    emit_pv_out(*prev)
                if qb == 0 and h + 1 < HPC:
                    q_nat_next = load_q(h + 1)
                    q_bf_next = cast_bf(h + 1, q_nat_next)
                    if qkT_dma:
                        transpose_64_dma(qT_tiles[(h + 1) % 2], q_bf_next)
                    else:
                        transpose_64(qT_tiles[(h + 1) % 2], q_bf_next, f"q{h + 1}_")
                prev = (idx, h, qb, pT)
            emit_pv_out(*prev)
    nc.compile()
    return nc


def _get_module(reps=1, **cfg):
    key = (reps, tuple(sorted((k, tuple(v) if isinstance(v, (list, tuple)) else v)
                              for k, v in cfg.items())))
    if key not in _CACHED:
        _CACHED[key] = _build_module(reps, **cfg)
    return _CACHED[key]


def make_in_maps(Q, K, V):
    """Shard full inputs into per-core input maps (core c -> batch c//4,
    heads 4*(c%4)..4*(c%4)+4)."""
    Q = np.asarray(Q, dtype=np.float32)
    K = np.asarray(K, dtype=np.float32)
    V = np.asarray(V, dtype=np.float32)
    in_maps = []
    for c in range(N_CORES):
        b = c // (N_CORES // B)
        h0 = HPC * (c % (N_CORES // B))
        in_maps.append(
            {
                "q": np.ascontiguousarray(Q[b, h0 : h0 + HPC]),
                "k": np.ascontiguousarray(K[b, 0]),
                "v": np.ascontiguousarray(V[b, 0]),
            }
        )
    return in_maps


def assemble_output(results):
    out = np.empty((B, H, S, D), dtype=np.float32)
    for c in range(N_CORES):
        b = c // (N_CORES // B)
        h0 = HPC * (c % (N_CORES // B))
        out[b, h0 : h0 + HPC] = results[c]["o"]
    return out


def kernel(Q, K, V):
    nc = _get_module(1, **DEFAULT_CFG)
    res = run_bass_kernel_spmd(nc, make_in_maps(Q, K, V), core_ids=list(range(N_CORES)))
    return assemble_output(res.results)


# revision 20
# speedup vs baseline: 1.4257x; 1.0054x over previous
"""MQA attention kernel for Trainium2 (8 NeuronCores, Bass/Tile).

Problem: Q [2,16,2048,64], K/V [2,1,2048,64] fp32, out = softmax(QK^T/8) V.

Sharding: 32 (batch, head) pairs over 8 cores -> 4 heads per core; each core
gets one batch's K/V (replicated across the 4 cores of that batch).

Per-core algorithm (S^T orientation so softmax reduction lands on the free dim
and PV needs no transposition of P):
  - Q/K are cast to bf16 on the Pool engine (idle otherwise), transposed with
    PE transpose-mode into K^T/Q^T [128, S] bf16 (d=64 zero-padded to 128;
    64-row stationaries measured slower, likely losing fast weight load).
  - S^T[j, q] = (K Q^T) in bf16 matmuls (1 cyc/row, FWL weight loads),
    one PSUM bank group per 1-2 j-chunks.
  - Score evacuation (exp(s/8) fused with PSUM->SBUF) is SPLIT between the
    ACT engine (real exp) and the DVE (Schraudolph approximate exp: one
    tensor_scalar computing int16(A*s + B) whose bit pattern IS bf16
    exp(s*scale) to ~+/-3%).  The split keeps the PE continuously fed so it
    stays at the 2.4 GHz p-state.  (GpSimd cannot access PSUM.)
  - PV uses bf16 V augmented with a ones column: one matmul chain yields both
    O^T = V'^T P^T and the softmax denominators (row 64).
  - Software pipelining: stage i's QK matmuls are emitted BEFORE stage i-1's
    PV matmuls (pT is triple-buffered), so evacuation engines always have
    score banks to drain while the PE streams PV.
  - O'^T evacuated to bf16, transposed back with PE, normalized with DVE
    reciprocal + per-partition multiply into fp32, DMA out.

The q-rows are processed in an interleaved order (partition p holds rows
16p+c) so every DMA moves contiguous 4KB runs; the same rearrange on the
output store undoes the permutation.

Measured on trn2 (NTFF profile): 149-153 us per core across all 8 cores
(baseline 213-215 us), rel err vs fp64-ish jax reference: 9.5e-3 (gate 2e-2).
PE active ~124 us of the span -- the QK+PV streaming floor for this dataflow
is ~109 us (each S^2-sized object crosses the PE's 128-lane PSUM/XBUS
interface once per matmul), so further gains need a different dataflow, not
scheduling.  Tried and measured WORSE: 64-row unpadded stationaries (+17us),
XBAR DMA transposes for the output path (+80us, SP HWDGE queue serializes),
9a/7d evac split (+25us), deferring the const-pool memsets (+24us).
"""

import math

import numpy as np

import concourse.mybir as mybir
import concourse.tile as tile
from concourse import bacc
from concourse.bass_utils import run_bass_kernel_spmd
from concourse.masks import make_identity

B, H, S, D = 2, 16, 2048, 64
N_CORES = 8
HPC = (B * H) // N_CORES  # heads per core = 4
P = 128
NJ = S // P               # 16 key chunks of 128
QB = 512                  # queries per block (= one PSUM bank of fp32 scores)
NQB = S // QB             # 4 q-blocks per head
SCALE = 1.0 / float(D) ** 0.5
F32 = mybir.dt.float32
BF16 = mybir.dt.bfloat16
I16 = mybir.dt.int16

# Schraudolph exp constants in bf16 (scale folded into A):
#   exp(s*SCALE) ~= bitcast_bf16(int16(A_SCH * s + B_SCH)), max rel err ~3%.
A_SCH = float(np.float32(2.0**7 / math.log(2.0) * SCALE))
B_SCH = float(np.float32(127 * 2**7 - 7.5))

# Per-qb score-evacuation schedule: tuple of (engine, chunk_count) covering
# the NJ=16 j-chunks in order. 'a' = ACT exp (exact), 'd' = DVE schraudolph.
DEFAULT_SCHED = ("d1", "a2", "d1", "a2", "d1", "a2", "d1", "a2", "d1", "a2", "d1")

_CACHED = {}
# Best measured config (profiled on HW); kernel() uses this.
DEFAULT_CFG = {}


def _build_module(reps=1, **cfg):
    nc = bacc.Bacc(None)
    q = nc.dram_tensor("q", [HPC, S, D], F32, kind="ExternalInput")
    k = nc.dram_tensor("k", [S, D], F32, kind="ExternalInput")
    v = nc.dram_tensor("v", [S, D], F32, kind="ExternalInput")
    o = nc.dram_tensor("o", [HPC, S, D], F32, kind="ExternalOutput")

    with tile.TileContext(nc) as tc:
        with tc.tile_pool(name="const", bufs=1) as cpool:
            identity = cpool.tile([P, P], BF16)
            make_identity(nc, identity)

            kT = cpool.tile([P, S], BF16)
            nc.gpsimd.memset(kT[64:P, :], 0.0)
            vp = cpool.tile([P, NJ, D + 1], BF16)
            nc.gpsimd.memset(vp[:, :, D], 1.0)
            qT_tiles = []
            for i in range(2):
                qTt = cpool.tile([P, S], BF16, name=f"qT{i}")
                nc.gpsimd.memset(qTt[64:P, :], 0.0)
                qT_tiles.append(qTt)

            for rep in range(reps):
                _trace_body(nc, tc, q, k, v, o, identity, kT, vp, qT_tiles, **cfg)
    nc.compile()
    return nc


def _trace_body(
    nc, tc, q, k, v, o, identity, kT, vp, qT_tiles,
    sched=None, act_bufs=2, s1_bufs=2, pt_bufs=3, tr_bufs=1, pv_bufs=1,
    norm_eng="v", qkT_dma=False, otr_dma=False,
):
    sched = sched if sched is not None else DEFAULT_SCHED
    groups = [(g[0], int(g[1:])) for g in sched]
    assert sum(gsz for _, gsz in groups) == NJ
    act_pad = max([gsz for eng, gsz in groups if eng == "a"] or [1])
    s1_pad = max([gsz for eng, gsz in groups if eng != "a"] or [1])

    with (
        tc.tile_pool(name="natb", bufs=2) as npool,
        tc.tile_pool(name="workb", bufs=2) as wpool,
        tc.tile_pool(name="ptb", bufs=pt_bufs) as ptpool,
        tc.tile_pool(name="psab", bufs=act_bufs, space="PSUM") as psa,
        tc.tile_pool(name="pssb", bufs=s1_bufs, space="PSUM") as pss,
        tc.tile_pool(name="ps1b", bufs=1, space="PSUM") as ps1,
    ):
            def cast_bf(h, src_nat, split=1):
                """Pool-engine fp32 -> bf16 cast (SBUF->SBUF; Pool is idle)."""
                bf = npool.tile([P, NJ, D], BF16, tag="bf", name=f"bf{h}")
                cs = NJ // split
                for i in range(split):
                    nc.gpsimd.tensor_copy(
                        bf[:, cs * i : cs * (i + 1), :],
                        src_nat[:, cs * i : cs * (i + 1), :],
                    )
                return bf

            def transpose_64_dma(dst, src_bf):
                # XBAR DMA transpose: [128, 64] bf16 chunks -> [64, 128] of
                # the destination, dispatched from the SP queue (no PE/DVE).
                for c in range(NJ):
                    nc.sync.dma_start_transpose(
                        dst[:, P * c : P * (c + 1)], src_bf[:, c, :]
                    )

            def transpose_64(dst, src_bf, who):
                # PE-transpose 4 [128,64] bf16 chunks into one PSUM tile, then
                # one DVE copy into [64, 512] of the bf16 destination.
                for g in range(NJ // 4):
                    pst = ps1.tile(
                        [64, 4, P], BF16, tag="tr", bufs=tr_bufs, name=f"pst_{who}{g}"
                    )
                    for t in range(4):
                        nc.tensor.transpose(
                            pst[:, t, :], src_bf[:, 4 * g + t, :], identity
                        )
                    nc.vector.tensor_copy(
                        dst[0:64, 512 * g : 512 * (g + 1)], pst[:]
                    )

            def load_q(h, split=1):
                q_nat = npool.tile([P, NJ, D], F32, tag="nat", name=f"q_nat{h}")
                qsrc = q[h].rearrange("(p c) d -> p c d", p=P)
                cs = NJ // split
                for i in range(split):
                    nc.sync.dma_start(
                        q_nat[:, cs * i : cs * (i + 1), :],
                        qsrc[:, cs * i : cs * (i + 1), :],
                    )
                return q_nat

            # ---- startup: K^T and head-0 Q^T, loads/casts/transposes all
            # chunk-interleaved so the first QK matmul can start early ----
            k_nat = npool.tile([P, NJ, D], F32, tag="nat")
            ksrc = k.rearrange("(p c) d -> p c d", p=P)
            for i in range(4):
                nc.sync.dma_start(
                    k_nat[:, 4 * i : 4 * (i + 1), :], ksrc[:, 4 * i : 4 * (i + 1), :]
                )
            q_nat0 = load_q(0, split=4)
            k_bf = cast_bf("k", k_nat, split=4)
            q_bf0 = cast_bf(0, q_nat0, split=4)
            for g in range(NJ // 4):
                pstk = ps1.tile([64, 4, P], BF16, tag="tr", bufs=tr_bufs, name=f"pst_k{g}")
                for t in range(4):
                    nc.tensor.transpose(pstk[:, t, :], k_bf[:, 4 * g + t, :], identity)
                nc.vector.tensor_copy(kT[0:64, 512 * g : 512 * (g + 1)], pstk[:])
                pstq = ps1.tile([64, 4, P], BF16, tag="tr", bufs=tr_bufs, name=f"pst_q0{g}")
                for t in range(4):
                    nc.tensor.transpose(pstq[:, t, :], q_bf0[:, 4 * g + t, :], identity)
                nc.vector.tensor_copy(
                    qT_tiles[0][0:64, 512 * g : 512 * (g + 1)], pstq[:]
                )

            # ---- V' [128, 16, 65] bf16: V plus a ones column ----
            v_nat = npool.tile([P, NJ, D], F32, tag="nat", name="v_nat")
            nc.sync.dma_start(v_nat[:], v.rearrange("(p c) d -> p c d", p=P))
            nc.gpsimd.tensor_copy(vp[:, :, 0:D], v_nat[:])

            def emit_qk(idx, h, qb):
                """QK^T matmuls for one 512-query block + 2-engine evacuation."""
                qT = qT_tiles[h % 2]
                qs = qT[:, QB * qb : QB * (qb + 1)]
                pT = ptpool.tile([P, NJ * QB], BF16, tag="pT", name=f"pT{idx % pt_bufs}")
                j0 = 0
                for gi, (eng, gsz) in enumerate(groups):
                    if eng == "a":
                        sg = psa.tile(
                            [P, gsz, QB], F32, tag="sga",
                            name=f"sga{idx}_{gi}", padded_shape=[P, act_pad, QB],
                        )
                    else:
                        sg = pss.tile(
                            [P, gsz, QB], F32, tag="sgs",
                            name=f"sgs{idx}_{gi}", padded_shape=[P, s1_pad, QB],
                        )
                    for i in range(gsz):
                        j = j0 + i
                        nc.tensor.matmul(
                            sg[:, i, :],
                            lhsT=kT[:, P * j : P * (j + 1)],
                            rhs=qs,
                            start=True,
                            stop=True,
                        )
                    dst = pT[:, QB * j0 : QB * (j0 + gsz)]
                    if eng == "a":
                        nc.scalar.activation(
                            dst, sg[:], mybir.ActivationFunctionType.Exp, scale=SCALE
                        )
                    else:
                        nc.vector.tensor_scalar(
                            dst.bitcast(I16), sg[:], A_SCH, B_SCH,
                            mybir.AluOpType.mult, mybir.AluOpType.add,
                        )
                    j0 += gsz
                return pT

            def emit_pv_out(idx, h, qb, pT):
                """PV accumulation, transpose back, normalize, store."""
                pv = ps1.tile([D + 1, QB], F32, tag="pv", bufs=pv_bufs, name=f"pv{idx}")
                for c in range(NJ):
                    nc.tensor.matmul(
                        pv[:],
                        lhsT=vp[:, c, :],
                        rhs=pT[:, QB * c : QB * (c + 1)],
                        start=(c == 0),
                        stop=(c == NJ - 1),
                    )
                rcp = wpool.tile([P, 4], F32, tag="rcp", name=f"rcp{idx}")
                oout = wpool.tile([P, 4, D], F32, tag="oout", name=f"oout{idx}")
                if otr_dma:
                    # O'^T evacuated into a full-128-partition bf16 tile (rows
                    # 65..127 zeroed for the XBAR), transposed back by the DMA
                    # XBAR in 128x128 blocks -- no PE/PSUM involved, and the
                    # normalize below reads all-SBUF operands (2x DVE mode).
                    oev = wpool.tile([P, QB], BF16, tag="oev", name=f"oev{idx}")
                    nc.vector.memset(oev[D : P, :], 0.0)
                    nc.vector.tensor_copy(oev[0 : D + 1, :], pv[:])
                    otr = wpool.tile([P, 4, P], BF16, tag="otr", name=f"otr{idx}")
                    for t in range(4):
                        nc.sync.dma_start_transpose(
                            otr[:, t, :], oev[:, P * t : P * (t + 1)]
                        )
                else:
                    oev = wpool.tile([D + 1, QB], BF16, tag="oev", name=f"oev{idx}")
                    nc.vector.tensor_copy(oev[:], pv[:])
                    # inner dim padded to D+2 so each bf16 PSUM chunk stays
                    # 4-byte aligned (66*2 = 132B)
                    otr = ps1.tile(
                        [P, 4, D + 2], BF16, tag="tr", bufs=tr_bufs, name=f"otr{idx}"
                    )
                    for t in range(4):
                        nc.tensor.transpose(
                            otr[:, t, 0 : D + 1],
                            oev[:, P * t : P * (t + 1)],
                            identity[0 : D + 1, 0 : D + 1],
                        )
                nc.vector.reciprocal(rcp[:], otr[:, :, D : D + 1])
                for t in range(4):
                    if norm_eng == "a":
                        nc.scalar.activation(
                            oout[:, t, :],
                            otr[:, t, 0:D],
                            mybir.ActivationFunctionType.Copy,
                            scale=rcp[:, t : t + 1],
                        )
                    else:
                        nc.vector.tensor_scalar(
                            oout[:, t, :],
                            otr[:, t, 0:D],
                            rcp[:, t : t + 1],
                            None,
                            mybir.AluOpType.mult,
                        )
                nc.sync.dma_start(
                    o[h].rearrange("(p c) d -> p c d", p=P)[
                        :, 4 * qb : 4 * (qb + 1), :
                    ],
                    oout[:],
                )

            stages = [(h, qb) for h in range(HPC) for qb in range(NQB)]
            prev = None
            for idx, (h, qb) in enumerate(stages):
                pT = emit_qk(idx, h, qb)
                if prev is not None:
                    emit_pv_out(*prev)
                if qb == 0 and h + 1 < HPC:
                    q_nat_next = load_q(h + 1)
                    q_bf_next = cast_bf(h + 1, q_nat_next)
                    if qkT_dma:
                        transpose_64_dma(qT_tiles[(h + 1) % 2], q_bf_next)
                    else:
                        transpose_64(qT_tiles[(h + 1) % 2], q_bf_next, f"q{h + 1}_")
                prev = (idx, h, qb, pT)
            emit_pv_out(*prev)
    nc.compile()
    return nc


def _get_module(reps=1, **cfg):
    key = (reps, tuple(sorted((k, tuple(v) if isinstance(v, (list, tuple)) else v)
                              for k, v in cfg.items())))
    if key not in _CACHED:
        _CACHED[key] = _build_module(reps, **cfg)
    return _CACHED[key]


def make_in_maps(Q, K, V):
    """Shard full inputs into per-core input maps (core c -> batch c//4,
    heads 4*(c%4)..4*(c%4)+4)."""
    Q = np.asarray(Q, dtype=np.float32)
    K = np.asarray(K, dtype=np.float32)
    V = np.asarray(V, dtype=np.float32)
    in_maps = []
    for c in range(N_CORES):
        b = c // (N_CORES // B)
        h0 = HPC * (c % (N_CORES // B))
        in_maps.append(
            {
                "q": np.ascontiguousarray(Q[b, h0 : h0 + HPC]),
                "k": np.ascontiguousarray(K[b, 0]),
                "v": np.ascontiguousarray(V[b, 0]),
            }
        )
    return in_maps


def assemble_output(results):
    out = np.empty((B, H, S, D), dtype=np.float32)
    for c in range(N_CORES):
        b = c // (N_CORES // B)
        h0 = HPC * (c % (N_CORES // B))
        out[b, h0 : h0 + HPC] = results[c]["o"]
    return out


def kernel(Q, K, V):
    nc = _get_module(1, **DEFAULT_CFG)
    res = run_bass_kernel_spmd(nc, make_in_maps(Q, K, V), core_ids=list(range(N_CORES)))
    return assemble_output(res.results)
